# revision 15
# baseline (speedup 1.0000x reference)
"""ComplexLayerScale Trainium2 kernel — tensor-engine formulation, fp16 I/O.

out[b,t,d] = (x_real + i*x_imag)[b,t,d] * (gamma_real + i*gamma_imag)[d]

Sharding: data-parallel over batch (B=8 -> 8 NeuronCores), gamma replicated.

Rel-err budget is 2e-2; fp16 rounding is ~5e-4, so all device I/O is fp16,
halving HBM traffic vs f32 (per core: 8.4 MB in + 8.4 MB out = 16.8 MB,
~47 us at the 360 GB/s per-core DMA ceiling). The f32 baseline was
DVE-bound at ~112 us busy; here the complex multiply runs on the (otherwise
idle) tensor engine and DVE/ACT only drain PSUM.

Layout: host transposes x to channel-major and packs per 64-channel chunk c
  xpack[c] = [xr rows c*64..c*64+63 ; xi rows ...]   # [128, T] fp16
so one 128x128 stationary weight per chunk
  W_c = [[diag(gr), diag(gi)], [diag(-gi), diag(gr)]]  # [K=128, M=128]
computes re (out partitions 0..63) and im (64..127) of 64 channels for all
T in one matmul pass: psum[m, t] = sum_k W[k, m] x[k, t]. PSUM (f32) is
copied to fp16 SBUF tiles (DVE/ACT alternating) and stored. Host unpacks
[c, comp, 64, T] fp16 -> [T, D] complex64 (exact widening).

Per chunk (1 MB in / 1 MB out): 2 strip loads (sync ring), 8 matmuls
(512 cols each = 1 PSUM bank), 8 copies, 2 strip stores (gpsimd ring).
Tiny warmer DMAs first: the first transfer on each HWDGE ring pays
~2.5-5 us of SDMA spin-up.
"""

import numpy as np

# Problem shape (hardcoded per contract).
B, T, D = 8, 4096, 512
N_CORES = 8
P = 128                    # SBUF partitions
NCHUNK = D // 64           # 8 chunks of 64 channels
NBANK = 512                # f32 elems per PSUM bank
STRIP = T // 2             # cols per load/store strip

_CACHE = {}


def _build_program():
    import concourse.bacc as bacc
    import concourse.bass as bass
    import concourse.mybir as mybir
    import concourse.tile as tile

    f16 = mybir.dt.float16
    f32 = mybir.dt.float32
    nc = bacc.Bacc("TRN2", target_bir_lowering=False, debug=False,
                   num_devices=N_CORES)

    xp = nc.dram_tensor("xp", [NCHUNK * P, T], f16, kind="ExternalInput")
    wt = nc.dram_tensor("wt", [P, NCHUNK * P], f16, kind="ExternalInput")
    y = nc.dram_tensor("y", [NCHUNK * P, T], f16, kind="ExternalOutput")

    with tile.TileContext(nc) as tc:
        with tc.tile_pool(name="w", bufs=1) as wpool, \
             tc.tile_pool(name="xa", bufs=NCHUNK) as xpa, \
             tc.tile_pool(name="xb", bufs=NCHUNK) as xpb, \
             tc.tile_pool(name="ya", bufs=3) as ypa, \
             tc.tile_pool(name="yb", bufs=3) as ypb, \
             tc.tile_pool(name="ps", bufs=4,
                          space=bass.MemorySpace.PSUM) as psp:

            # Ring warmers: each DGE ring pays ~2.5-5us spin-up on its first
            # transfer; burn it on 4 bytes. Only sync(SP)/scalar(ACT)/gpsimd
            # can initiate DMAs. The aggregate DMA ceiling (~400 GB/s/core)
            # binds, not per-queue caps, so queue assignment only shapes the
            # ramp and tail; the end-of-NEFF event teardown costs ~115ns per
            # DMA instruction on the PE sequencer, so fewer+bigger DMAs win:
            # 1 MB chunk loads alternate sync/scalar, 1 MB stores ride
            # gpsimd, and the last chunks' stores go as 0.5 MB strips on
            # sync/scalar after their loads have drained.
            # Queue discipline, learned the hard way (58->72us when
            # violated): queues are FIFO, so a queue's stores cannot begin
            # until its own loads fully drain, and aggregate DMA is pinned
            # at ~420 GB/s whatever the queue count. So: ALL loads ride
            # sync (free-runs at ~380 GB/s solo, shares ~210 once stores
            # flow); stores split gpsimd (even chunks) / scalar (odd) so
            # they chase the loads from t~13 with no FIFO coupling. All of
            # x fits in SBUF (bufs=NCHUNK): every load is emitted up-front
            # with no buffer-reuse waits.
            for i, eng in enumerate((nc.sync, nc.scalar, nc.gpsimd)):
                wi = wpool.tile([1, 1], f16, tag=f"warm_in{i}")
                eng.dma_start(out=wi[:], in_=wt[0:1, 0:1])

            wsb = wpool.tile([P, NCHUNK * P], f16, tag="w")
            nc.scalar.dma_start(out=wsb[:], in_=wt[:])

            all_xs = []
            for c in range(NCHUNK):
                r0 = c * P
                xs = []
                for s, pool in ((0, xpa), (1, xpb)):
                    xt = pool.tile([P, STRIP], f16, tag=f"x{s}")
                    nc.sync.dma_start(
                        out=xt[:],
                        in_=xp[r0:r0 + P, s * STRIP:(s + 1) * STRIP])
                    xs.append(xt)
                all_xs.append(xs)

            # PSUM->SBUF drain splits 3:1 DVE:ACT (GPSIMD cannot access
            # PSUM; ACT also dispatches DMAs); 1024-col copies span two
            # PSUM banks (each matmul output stays within one bank).
            copy_engs = [lambda o, i_: nc.vector.tensor_copy(o, i_),
                         lambda o, i_: nc.scalar.copy(o, i_)]

            def store_eng(c, s):
                return nc.gpsimd if c % 2 == 0 else nc.scalar

            for c in range(NCHUNK):
                r0 = c * P
                wc = wsb[:, c * P:(c + 1) * P]
                xs = all_xs[c]
                for s, pool in ((0, ypa), (1, ypb)):
                    yt = pool.tile([P, STRIP], f16, tag=f"y{s}")
                    for h in range(STRIP // (2 * NBANK)):
                        ps = psp.tile([P, 2 * NBANK], f32, tag="ps")
                        for jj in range(2):
                            jo = (2 * h + jj) * NBANK
                            nc.tensor.matmul(
                                ps[:, jj * NBANK:(jj + 1) * NBANK], wc,
                                xs[s][:, jo:jo + NBANK],
                                start=True, stop=True)
                        copy_engs[1 if (s, h) == (1, 1) else 0](
                            yt[:, h * 2 * NBANK:(h + 1) * 2 * NBANK], ps[:])
                    store_eng(c, s).dma_start(
                        out=y[r0:r0 + P, s * STRIP:(s + 1) * STRIP],
                        in_=yt[:])
    nc.compile()
    return nc


def _get_program():
    if "nc" not in _CACHE:
        _CACHE["nc"] = _build_program()
    return _CACHE["nc"]


def _weights(gamma_real, gamma_imag):
    gr = np.asarray(gamma_real, dtype=np.float32)
    gi = np.asarray(gamma_imag, dtype=np.float32)
    w = np.zeros((NCHUNK, 2, 64, 2, 64), dtype=np.float32)  # [c,kb,k,mb,m]
    idx = np.arange(64)
    for c in range(NCHUNK):
        grc, gic = gr[c * 64:(c + 1) * 64], gi[c * 64:(c + 1) * 64]
        w[c, 0, idx, 0, idx] = grc
        w[c, 0, idx, 1, idx] = gic
        w[c, 1, idx, 0, idx] = -gic
        w[c, 1, idx, 1, idx] = grc
    # [c, k, m] -> [k, c*128 + m]
    wt = w.reshape(NCHUNK, P, P).transpose(1, 0, 2).reshape(P, NCHUNK * P)
    return np.ascontiguousarray(wt.astype(np.float16))


def _pack_x(x_real, x_imag):
    xr = np.asarray(x_real, dtype=np.float32)
    xi = np.asarray(x_imag, dtype=np.float32)
    xp = np.empty((B, NCHUNK, 2, 64, T), dtype=np.float16)
    xp[:, :, 0] = xr.reshape(B, T, NCHUNK, 64).transpose(0, 2, 3, 1)
    xp[:, :, 1] = xi.reshape(B, T, NCHUNK, 64).transpose(0, 2, 3, 1)
    return xp.reshape(B, NCHUNK * P, T)


def _in_maps(x_real, x_imag, gamma_real, gamma_imag):
    wt = _weights(gamma_real, gamma_imag)
    xp = _pack_x(x_real, x_imag)
    return [{"xp": xp[b], "wt": wt} for b in range(N_CORES)]


def _unpack_y(res):
    yall = np.stack([res.results[c]["y"] for c in range(N_CORES)], axis=0)
    yv = yall.reshape(B, NCHUNK, 2, 64, T)
    out = np.empty((B, T, D), dtype=np.complex64)
    of = out.view(np.float32).reshape(B, T, NCHUNK, 64, 2)
    of[...] = yv.transpose(0, 4, 1, 3, 2)  # [B, T, c, ch, comp]
    return out


def kernel(x_real, x_imag, gamma_real, gamma_imag):
    from concourse.bass_utils import run_bass_kernel_spmd

    nc = _get_program()
    res = run_bass_kernel_spmd(
        nc, _in_maps(x_real, x_imag, gamma_real, gamma_imag),
        list(range(N_CORES)))
    return _unpack_y(res)


def run_traced(x_real, x_imag, gamma_real, gamma_imag, **kw):
    """Profiled run (for test.py): returns BassKernelResults with
    exec_time_ns populated from the NTFF profile."""
    from concourse.bass_utils import run_bass_kernel_spmd

    nc = _get_program()
    return run_bass_kernel_spmd(
        nc, _in_maps(x_real, x_imag, gamma_real, gamma_imag),
        list(range(N_CORES)), trace=True, **kw)
